# revision 41
# baseline (speedup 1.0000x reference)
"""CBTree bottom-up fold kernel for 8 trn2 NeuronCores.

Problem: complete 4-ary tree, 9 levels, 87381 nodes in BFS order, d=256.
  leaves (level 8): h = vectors[21845:]
  internal node:    h = tanh(sum_i W_i @ h_child_i + vectors[node])
  where W_i = lc[i]*Wl + rc[i]*Wr,  lc=[1,2/3,1/3,0], rc=[0,1/3,2/3,1].

Strategy (data-parallel over sibling groups):
  - Shard every level contiguously over 8 cores. Children of a core's
    parents are exactly the core's own previous-level outputs, so levels
    7..2 run with zero communication.
  - One AllGather of the level-2 states (16 nodes, 1KB/rank fp16), then
    every core redundantly folds levels 1..0 and writes the root.  The
    collective carries a ~15us fixed cost in this stack, so exactly one
    is issued; gathering level 2 (not 3) minimizes both the payload and
    the post-collective tail.
  - On chip h lives transposed ([128, n] tiles, one per 128-row d-half)
    so the tensor engine contracts over d; the host-side sharding step
    hands each core its slices already in this layout (a zero-FLOP
    relayout done while slicing).  Level l is 8 accumulating 128x128xN
    matmuls per output half (4 sibling positions x 2 d-halves).
  - The bias vector is added by the DVE (scalar_tensor_tensor on PSUM)
    for the two big levels (7,6), freeing the tensor engine of the
    identity-matmul bias; small levels and the tail keep the identity
    matmul (lower latency, negligible PE load).
  - Leaf + bias DMAs are chunk-interleaved with level-7 compute;
    level-6 chunks are emitted in a wavefront between level-7 chunks so
    the PE never idles while the leaf stream finishes.  All SBUF tiles
    are 2-D with column-slice chunking so Tile tracks regions exactly.
  - A few dummy matmuls at t~0 ramp the PE out of its slow p-state
    before real work arrives.
  - fp16 throughout (fp32 PSUM accumulation): halves every DMA stream
    vs fp32 and runs the PE at full rate; ~3.4e-3 scale-relative error.
"""

import numpy as np

F32 = None  # set on first _lazy_imports()

_BASS = {}


def _lazy_imports():
    global bass, bacc, mybir, tile, make_identity, run_bass_kernel_spmd, F32
    import concourse.bass as bass
    import concourse.mybir as mybir
    from concourse import bacc
    import concourse.tile as tile
    from concourse.masks import make_identity
    from concourse.bass_utils import run_bass_kernel_spmd
    F32 = mybir.dt.float32


N_CORES = 8
D = 256
B = 4
L = 9
SIZES = [B**l for l in range(L)]            # [1,4,16,64,256,1024,4096,16384,65536]
OFFSETS = np.concatenate([[0], np.cumsum(SIZES)])  # [...,21845,87381]
N_LEAF_CORE = SIZES[8] // N_CORES           # 8192
LOC_LEVELS = [7, 6, 5, 4, 3, 2]
LOC_PAR = {l: SIZES[l] // N_CORES for l in LOC_LEVELS}  # 2048,512,128,32,8,2
N_VECS_LOC = sum(LOC_PAR.values())          # 2730
# vtail: 4 level-1 vectors then the root vector replicated 4x
N_VECS_TAIL = 8


def _build_nc(mode="fp16", probe=None, WARM=6, L7CHUNK=256):
    key = ("nc", mode, probe, WARM, L7CHUNK)
    if key in _BASS:
        return _BASS[key]
    nc = bacc.Bacc(num_devices=N_CORES)
    mmdt = {"fp32": F32, "fp16": mybir.dt.float16}[mode]
    Tanh = mybir.ActivationFunctionType.Tanh

    # all h/vec tensors arrive transposed/packed: [128 d', 2, n]
    leavesT = nc.declare_dram_parameter("leavesT", [128, 2, N_LEAF_CORE], mmdt,
                                        isOutput=False)
    vecs_locT = nc.declare_dram_parameter("vecs_locT", [128, 2, N_VECS_LOC],
                                          mmdt, isOutput=False)
    vecs_tailT = nc.declare_dram_parameter("vecs_tailT", [128, 2, N_VECS_TAIL],
                                           mmdt, isOutput=False)
    wmat = nc.declare_dram_parameter("wmat", [128, 16 * 128], mmdt,
                                     isOutput=False)
    out = nc.declare_dram_parameter("out", [1, D], F32, isOutput=True)

    N_L7CHUNKS = LOC_PAR[7] // L7CHUNK                    # 8 at L7CHUNK=256
    LEAF_DMA = 4 * L7CHUNK                                # leaf cols per chunk

    with tile.TileContext(nc) as tc:
        with (
            tc.tile_pool(name="const", bufs=1) as const_pool,
            tc.tile_pool(name="hbuf", bufs=1) as hbuf,
            tc.tile_pool(name="pmm", bufs=8, space="PSUM") as psum_mm,
            tc.tile_pool(name="dram", bufs=1, space="DRAM") as dram_pool,
        ):
            # --- PE p-state warmup: dummy matmuls on a memset tile ---
            warm_rhs = const_pool.tile([128, 512], mmdt, name="warm_rhs")
            nc.gpsimd.memset(warm_rhs[:], 0.0)
            for w in range(WARM):
                ps_w = psum_mm.tile([128, 512], F32, name="ps_w", tag="mm")
                nc.tensor.matmul(ps_w[:, :512], warm_rhs[:, :128],
                                 warm_rhs[:], start=True, stop=True)

            wsb = const_pool.tile([128, 16 * 128], mmdt, name="wsb")
            ident = const_pool.tile([128, 128], mmdt, name="ident")
            make_identity(nc, ident)
            # touch Tanh once so the activation-table load happens during
            # the initial DMA shadow instead of before the first real tanh
            warm_act = const_pool.tile([128, 4], F32, name="warm_act")
            nc.scalar.activation(warm_act[:1, :4], ident[:1, :4], Tanh)

            # persistent h states: one 2-D tile per (level, d-half)
            def h_tiles(name, n):
                return [hbuf.tile([128, max(n, 2)], mmdt, name=f"{name}_{kh}",
                                  tag=f"{name}_{kh}") for kh in (0, 1)]

            hT = {8: h_tiles("hT8", N_LEAF_CORE)}
            for l in (7, 6, 5, 4, 3):
                hT[l] = h_tiles(f"hT{l}", LOC_PAR[l])
            # level-2 / tail states packed (both halves in one tile) so the
            # bounce / store is one DMA
            NLOC = LOC_PAR[2]                                # 2
            t2p = hbuf.tile([128, 2 * NLOC], mmdt, name="t2p", tag="t2p")
            hT[2] = [t2p[:, 0:NLOC], t2p[:, NLOC:2 * NLOC]]
            h2all = h_tiles("h2all", SIZES[2])               # [128, 16] x2
            h1p = h_tiles("h1p", SIZES[1])                   # [128, 4] x2
            t0p = hbuf.tile([128, 8], F32, name="t0p", tag="t0p")
            t0 = [t0p[:, 0:4], t0p[:, 4:8]]

            vloc = h_tiles("vloc", N_VECS_LOC)               # [128, 2730] x2 (mh)
            vtail = hbuf.tile([128, 2, N_VECS_TAIL], mmdt, name="vtail",
                              tag="vtail")

            # --- DMA stream (sync queue order = issue order) ---
            def leaf_dma(a, b):
                if probe == "nodma":
                    if a == 0:
                        for kh in (0, 1):
                            nc.gpsimd.memset(hT[8][kh][:], 0.0)
                    return
                for kh in (0, 1):
                    nc.sync.dma_start(hT[8][kh][:, a:b], leavesT[:, kh, a:b])

            def vloc_dma(a, b):
                for mh in (0, 1):
                    nc.sync.dma_start(vloc[mh][:, a:b], vecs_locT[:, mh, a:b])

            # --- shared level routine ---
            # vcol0: column offset of this level's bias vectors in vloc
            VCOL0 = {}
            _c = 0
            for l in LOC_LEVELS:
                VCOL0[l] = _c
                _c += LOC_PAR[l]

            def do_chunk(l, c0, N, bias_dve, vec=None, vcol0=None,
                         rview=None, hout=None, khs=(0, 1)):
                if vcol0 is None:
                    vcol0 = VCOL0[l]
                if hout is None:
                    hout = hT[l]
                if rview is None:
                    n_par = LOC_PAR[l]
                    rview = [hT[l + 1][kh][:, :4 * n_par]
                             .rearrange("k (p four) -> k p four", four=4)
                             for kh in (0, 1)]
                for mh in (0, 1):
                    ps = psum_mm.tile([128, 512], F32, name="ps_mm", tag="mm")
                    vts = (vloc[mh][:, vcol0 + c0: vcol0 + c0 + N]
                           if vec is None else vec[:, mh, vcol0 + c0:vcol0 + c0 + N])
                    if not bias_dve:
                        # bias first: opens the PSUM group before the child
                        # activations land (vectors are preloaded)
                        nc.tensor.matmul(ps[:, :N], ident[:], vts,
                                         start=True, stop=False)
                    for kh in khs:
                        for i in range(4):
                            blk = mh * 8 + i * 2 + kh
                            w = wsb[:, blk * 128:(blk + 1) * 128]
                            rhs = rview[kh][:, c0:c0 + N, i]
                            nc.tensor.matmul(
                                ps[:, :N], w, rhs,
                                start=(bias_dve and i == 0 and kh == khs[0]),
                                stop=(i == 3 and kh == khs[1]))
                    if bias_dve:
                        nc.vector.scalar_tensor_tensor(
                            ps[:, :N], ps[:, :N], 1.0, vts,
                            op0=mybir.AluOpType.mult, op1=mybir.AluOpType.add)
                    nc.scalar.activation(hout[mh][:, c0:c0 + N], ps[:, :N], Tanh)

            # --- DMA stream plan (issue order; deps let HWDGE pipeline) ---
            # leaves in 5 slabs (small first for an early PE start), bias
            # vectors in 3 consolidated slabs issued where consumption allows
            nc.sync.dma_start(hT[8][0][:, 0:1024], leavesT[:, 0, 0:1024])
            nc.sync.dma_start(wsb[:, :8 * 128], wmat[:, :8 * 128])  # mh0 blocks
            nc.sync.dma_start(hT[8][1][:, 0:1024], leavesT[:, 1, 0:1024])
            nc.sync.dma_start(wsb[:, 8 * 128:], wmat[:, 8 * 128:])  # mh1 blocks
            leaf_dma(1024, 3072)                   # chunks 1-2
            vloc_dma(0, 768)                       # bias, chunks 0-2
            leaf_dma(3072, 5120)                   # chunks 3-4
            vloc_dma(768, 2048)                    # bias, chunks 3-7
            leaf_dma(5120, 7168)                   # chunks 5-6
            leaf_dma(7168, 8192)                   # chunk 7
            vloc_dma(2048, N_VECS_LOC)             # bias, levels 6..2

            if probe == "A":
                nc.sync.dma_start(out[0:1, 0:16], hT[8][0][:1, :32].bitcast(F32))
            else:
                # level-6 chunks of 128 interleave into the level-7 wavefront
                for c in range(N_L7CHUNKS):
                    do_chunk(7, c * L7CHUNK, L7CHUNK, bias_dve=True)
                    if c >= 3 and c % 2 == 1:      # after c=3,5,7
                        do_chunk(6, 128 * (c - 3) // 2, 128, bias_dve=True)
                do_chunk(6, 384, 128, bias_dve=True)
                # half-chunked cascade: level l's first half depends only on
                # the first half of level l+1, so the chains pipeline
                do_chunk(5, 0, 64, bias_dve=False)
                do_chunk(5, 64, 64, bias_dve=False)
                do_chunk(4, 0, 16, bias_dve=False)
                do_chunk(4, 16, 16, bias_dve=False)
                do_chunk(3, 0, 4, bias_dve=False)
                do_chunk(3, 4, 4, bias_dve=False)
                do_chunk(2, 0, 2, bias_dve=False)

            if probe == "B":
                nc.sync.dma_start(out[0:1, 0:1], t2p[:1, :2].bitcast(F32))
            elif probe != "A":
                # --- AllGather of the level-2 states ---
                cc_in = dram_pool.tile([D, NLOC], mmdt, name="cc_in")
                cc_out = dram_pool.tile([N_CORES * D, NLOC], mmdt,
                                        name="cc_out")
                nc.sync.dma_start(
                    cc_in[:].rearrange("(kh k) n -> k kh n", kh=2),
                    t2p[:].rearrange("k (kh n) -> k kh n", kh=2))
                nc.sync.dma_start(vtail[:], vecs_tailT[:])    # hides under cc
                nc.gpsimd.collective_compute(
                    "AllGather", mybir.AluOpType.bypass,
                    replica_groups=[list(range(N_CORES))],
                    ins=[cc_in.opt()], outs=[cc_out.opt()])
                cc_v = cc_out[:].rearrange("(r kh k) n -> kh k r n",
                                           r=N_CORES, kh=2)
                for kh in (1, 0):
                    nc.sync.dma_start(
                        h2all[kh][:].rearrange("k (r n) -> k r n",
                                               r=N_CORES), cc_v[kh])

                if probe == "C":
                    nc.sync.dma_start(out[0:1, 0:8],
                                      h2all[0][:1, :16].bitcast(F32))
                else:
                    # --- replicated tail: level 1, then the root (x4) ---
                    # kh1 fetched first; kh1 matmuls lead so the kh0 fetch
                    # hides under them
                    rv1 = [h2all[kh][:, :SIZES[2]]
                           .rearrange("k (p four) -> k p four", four=4)
                           for kh in (0, 1)]
                    do_chunk(1, 0, 4, bias_dve=False, vec=vtail,
                             vcol0=0, rview=rv1, hout=h1p, khs=(1, 0))
                    rv0 = [h1p[kh][:, 0:4].unsqueeze(1)
                           .broadcast_to([128, 4, 4]) for kh in (0, 1)]
                    do_chunk(0, 0, 4, bias_dve=False, vec=vtail,
                             vcol0=4, rview=rv0,
                             hout=[t0p[:, 0:4], t0p[:, 4:8]])
                    # root natural-layout store: [128,2] partition-scatter
                    nc.sync.dma_start(
                        out[:].rearrange("o (mh m) -> m (o mh)", mh=2),
                        t0p[:].rearrange("k (mh n) -> k mh n", mh=2)[:, :, 0])

    nc.finalize()
    _BASS[key] = nc
    return nc


def _prep_inputs(vectors, Wl, Wr, mode="fp16"):
    vectors = np.asarray(vectors, dtype=np.float32)
    Wl = np.asarray(Wl, dtype=np.float32)
    Wr = np.asarray(Wr, dtype=np.float32)

    ind = np.arange(1, B + 1, dtype=np.float32)
    lc = (B - ind) / (B - 1)
    rc = (ind - 1) / (B - 1)
    # W_t[i] = W_i.T laid out [k', (mh, i, kh) blocks] for SBUF [128, 2048]
    Wt = np.stack([lc[i] * Wl.T + rc[i] * Wr.T for i in range(B)])  # [4,256k,256m]
    W5 = Wt.reshape(4, 2, 128, 2, 128)            # [i, kh, k', mh, m']
    blocks = [W5[i, kh, :, mh, :]
              for mh in (0, 1) for i in range(4) for kh in (0, 1)]
    wmat = np.ascontiguousarray(np.concatenate(blocks, axis=1),
                                dtype=np.float32)

    # one transposed copy of the node array; per-core slices are views into
    # it laid out [128, 2, n] (part of sharding, no arithmetic)
    vecsT = np.ascontiguousarray(vectors.T)                      # [256, 87381]
    vP = vecsT.reshape(2, 128, vecsT.shape[1])                   # [mh/kh,128,n]

    def packed(cols):
        return np.ascontiguousarray(vP[:, :, cols].transpose(1, 0, 2))

    tail_cols = np.array([1, 2, 3, 4, 0, 0, 0, 0])
    vecs_tailT = packed(tail_cols)

    hdt = np.float16 if mode == "fp16" else np.float32
    in_maps = []
    for c in range(N_CORES):
        o8 = int(OFFSETS[8])
        leaf_cols = slice(o8 + c * N_LEAF_CORE, o8 + (c + 1) * N_LEAF_CORE)
        loc_parts = []
        for l in LOC_LEVELS:
            npl = LOC_PAR[l]
            o = int(OFFSETS[l])
            loc_parts.append(vP[:, :, o + c * npl: o + (c + 1) * npl])
        vloc_c = np.ascontiguousarray(
            np.concatenate(loc_parts, axis=2).transpose(1, 0, 2))
        im = {
            "leavesT": packed(leaf_cols).astype(hdt),
            "vecs_locT": vloc_c.astype(hdt),
            "vecs_tailT": vecs_tailT.astype(hdt),
            "wmat": wmat.astype(hdt),
        }
        in_maps.append(im)
    return in_maps


def kernel(vectors, Wl, Wr, branching, n_levels, _mode="fp16"):
    _lazy_imports()
    assert int(branching) == B and int(n_levels) == L
    vectors = np.asarray(vectors)
    assert vectors.shape == (int(OFFSETS[L]), D), vectors.shape

    nc = _build_nc(mode=_mode)
    in_maps = _prep_inputs(vectors, Wl, Wr, mode=_mode)
    try:
        res = run_bass_kernel_spmd(nc, in_maps, core_ids=list(range(N_CORES)),
                                   trace=False)
    except Exception:
        # transient device hiccups (e.g. NRT_EXEC_UNIT_UNRECOVERABLE right
        # after another process released the cores) clear on a retry
        res = run_bass_kernel_spmd(nc, in_maps, core_ids=list(range(N_CORES)),
                                   trace=False)
    root = res.results[0]["out"]
    return np.asarray(root, dtype=np.float32).reshape(1, D)


# revision 52
# speedup vs baseline: 1.0054x; 1.0054x over previous
"""CBTree bottom-up fold kernel for 8 trn2 NeuronCores.

Problem: complete 4-ary tree, 9 levels, 87381 nodes in BFS order, d=256.
  leaves (level 8): h = vectors[21845:]
  internal node:    h = tanh(sum_i W_i @ h_child_i + vectors[node])
  where W_i = lc[i]*Wl + rc[i]*Wr,  lc=[1,2/3,1/3,0], rc=[0,1/3,2/3,1].

Strategy (data-parallel over sibling groups):
  - Shard every level contiguously over 8 cores. Children of a core's
    parents are exactly the core's own previous-level outputs, so levels
    7..2 run with zero communication.
  - One AllGather of the level-2 states (16 nodes, 1KB/rank fp16), then
    every core redundantly folds levels 1..0 and writes the root.  The
    collective carries a ~15us fixed cost in this stack, so exactly one
    is issued; gathering level 2 (not 3) minimizes both the payload and
    the post-collective tail.
  - On chip h lives transposed ([128, n] tiles, one per 128-row d-half)
    so the tensor engine contracts over d; the host-side sharding step
    hands each core its slices already in this layout (a zero-FLOP
    relayout done while slicing).  Level l is 8 accumulating 128x128xN
    matmuls per output half (4 sibling positions x 2 d-halves).
  - The bias vector is added by the DVE (scalar_tensor_tensor on PSUM)
    for the two big levels (7,6), freeing the tensor engine of the
    identity-matmul bias; small levels and the tail keep the identity
    matmul (lower latency, negligible PE load).
  - Leaf + bias DMAs are chunk-interleaved with level-7 compute;
    level-6 chunks are emitted in a wavefront between level-7 chunks so
    the PE never idles while the leaf stream finishes.  All SBUF tiles
    are 2-D with column-slice chunking so Tile tracks regions exactly.
  - A few dummy matmuls at t~0 ramp the PE out of its slow p-state
    before real work arrives.
  - fp16 throughout (fp32 PSUM accumulation): halves every DMA stream
    vs fp32 and runs the PE at full rate; ~3.4e-3 scale-relative error.
"""

import numpy as np

F32 = None  # set on first _lazy_imports()

_BASS = {}


def _lazy_imports():
    global bass, bacc, mybir, tile, make_identity, run_bass_kernel_spmd, F32
    import concourse.bass as bass
    import concourse.mybir as mybir
    from concourse import bacc
    import concourse.tile as tile
    from concourse.masks import make_identity
    from concourse.bass_utils import run_bass_kernel_spmd
    F32 = mybir.dt.float32


N_CORES = 8
D = 256
B = 4
L = 9
SIZES = [B**l for l in range(L)]            # [1,4,16,64,256,1024,4096,16384,65536]
OFFSETS = np.concatenate([[0], np.cumsum(SIZES)])  # [...,21845,87381]
N_LEAF_CORE = SIZES[8] // N_CORES           # 8192
LOC_LEVELS = [7, 6, 5, 4, 3, 2]
LOC_PAR = {l: SIZES[l] // N_CORES for l in LOC_LEVELS}  # 2048,512,128,32,8,2
N_VECS_LOC = sum(LOC_PAR.values())          # 2730
# vtail: 4 level-1 vectors then the root vector replicated 4x
N_VECS_TAIL = 8


def _build_nc(mode="fp16", probe=None, WARM=6, L7CHUNK=256):
    key = ("nc", mode, probe, WARM, L7CHUNK)
    if key in _BASS:
        return _BASS[key]
    nc = bacc.Bacc(num_devices=N_CORES)
    mmdt = {"fp32": F32, "fp16": mybir.dt.float16}[mode]
    Tanh = mybir.ActivationFunctionType.Tanh

    # all h/vec tensors arrive transposed/packed: [128 d', 2, n]
    leavesT = nc.declare_dram_parameter("leavesT", [128, 2, N_LEAF_CORE], mmdt,
                                        isOutput=False)
    vecs_locT = nc.declare_dram_parameter("vecs_locT", [128, 2, N_VECS_LOC],
                                          mmdt, isOutput=False)
    vecs_tailT = nc.declare_dram_parameter("vecs_tailT", [128, 2, N_VECS_TAIL],
                                           mmdt, isOutput=False)
    wmat = nc.declare_dram_parameter("wmat", [128, 16 * 128], mmdt,
                                     isOutput=False)
    out = nc.declare_dram_parameter("out", [1, D], F32, isOutput=True)

    N_L7CHUNKS = LOC_PAR[7] // L7CHUNK                    # 8 at L7CHUNK=256
    LEAF_DMA = 4 * L7CHUNK                                # leaf cols per chunk

    with tile.TileContext(nc) as tc:
        with (
            tc.tile_pool(name="const", bufs=1) as const_pool,
            tc.tile_pool(name="hbuf", bufs=1) as hbuf,
            tc.tile_pool(name="pmm", bufs=8, space="PSUM") as psum_mm,
            tc.tile_pool(name="dram", bufs=1, space="DRAM") as dram_pool,
        ):
            # --- PE p-state warmup: dummy matmuls on a memset tile ---
            warm_rhs = const_pool.tile([128, 512], mmdt, name="warm_rhs")
            nc.gpsimd.memset(warm_rhs[:], 0.0)
            for w in range(WARM):
                ps_w = psum_mm.tile([128, 512], F32, name="ps_w", tag="mm")
                nc.tensor.matmul(ps_w[:, :512], warm_rhs[:, :128],
                                 warm_rhs[:], start=True, stop=True)

            wsb = const_pool.tile([128, 16 * 128], mmdt, name="wsb")
            ident = const_pool.tile([128, 128], mmdt, name="ident")
            make_identity(nc, ident)
            # touch Tanh once so the activation-table load happens during
            # the initial DMA shadow instead of before the first real tanh
            warm_act = const_pool.tile([128, 4], F32, name="warm_act")
            nc.scalar.activation(warm_act[:1, :4], ident[:1, :4], Tanh)

            # persistent h states: one 2-D tile per (level, d-half)
            def h_tiles(name, n):
                return [hbuf.tile([128, max(n, 2)], mmdt, name=f"{name}_{kh}",
                                  tag=f"{name}_{kh}") for kh in (0, 1)]

            hT = {8: h_tiles("hT8", N_LEAF_CORE)}
            for l in (7, 6, 5, 4, 3):
                hT[l] = h_tiles(f"hT{l}", LOC_PAR[l])
            # level-2 / tail states packed (both halves in one tile) so the
            # bounce / store is one DMA
            NLOC = LOC_PAR[2]                                # 2
            t2p = hbuf.tile([128, 2 * NLOC], mmdt, name="t2p", tag="t2p")
            hT[2] = [t2p[:, 0:NLOC], t2p[:, NLOC:2 * NLOC]]
            h2all = h_tiles("h2all", SIZES[2])               # [128, 16] x2
            h1p = h_tiles("h1p", SIZES[1])                   # [128, 4] x2
            t0p = hbuf.tile([128, 8], F32, name="t0p", tag="t0p")
            t0 = [t0p[:, 0:4], t0p[:, 4:8]]

            vloc = h_tiles("vloc", N_VECS_LOC)               # [128, 2730] x2 (mh)
            vtail = hbuf.tile([128, 2, N_VECS_TAIL], mmdt, name="vtail",
                              tag="vtail")

            # --- DMA stream (sync queue order = issue order) ---
            def leaf_dma(a, b):
                if probe == "nodma":
                    if a == 0:
                        for kh in (0, 1):
                            nc.gpsimd.memset(hT[8][kh][:], 0.0)
                    return
                for kh in (0, 1):
                    nc.sync.dma_start(hT[8][kh][:, a:b], leavesT[:, kh, a:b])

            def vloc_dma(a, b):
                for mh in (0, 1):
                    nc.sync.dma_start(vloc[mh][:, a:b], vecs_locT[:, mh, a:b])

            # --- shared level routine ---
            # vcol0: column offset of this level's bias vectors in vloc
            VCOL0 = {}
            _c = 0
            for l in LOC_LEVELS:
                VCOL0[l] = _c
                _c += LOC_PAR[l]

            def do_chunk(l, c0, N, bias_dve, vec=None, vcol0=None,
                         rview=None, hout=None, khs=(0, 1)):
                if vcol0 is None:
                    vcol0 = VCOL0[l]
                if hout is None:
                    hout = hT[l]
                if rview is None:
                    n_par = LOC_PAR[l]
                    rview = [hT[l + 1][kh][:, :4 * n_par]
                             .rearrange("k (p four) -> k p four", four=4)
                             for kh in (0, 1)]
                for mh in (0, 1):
                    ps = psum_mm.tile([128, 512], F32, name="ps_mm", tag="mm")
                    vts = (vloc[mh][:, vcol0 + c0: vcol0 + c0 + N]
                           if vec is None else vec[:, mh, vcol0 + c0:vcol0 + c0 + N])
                    if not bias_dve:
                        # bias first: opens the PSUM group before the child
                        # activations land (vectors are preloaded)
                        nc.tensor.matmul(ps[:, :N], ident[:], vts,
                                         start=True, stop=False)
                    for kh in khs:
                        for i in range(4):
                            blk = mh * 8 + i * 2 + kh
                            w = wsb[:, blk * 128:(blk + 1) * 128]
                            rhs = rview[kh][:, c0:c0 + N, i]
                            nc.tensor.matmul(
                                ps[:, :N], w, rhs,
                                start=(bias_dve and i == 0 and kh == khs[0]),
                                stop=(i == 3 and kh == khs[1]))
                    if bias_dve:
                        nc.vector.scalar_tensor_tensor(
                            ps[:, :N], ps[:, :N], 1.0, vts,
                            op0=mybir.AluOpType.mult, op1=mybir.AluOpType.add)
                    nc.scalar.activation(hout[mh][:, c0:c0 + N], ps[:, :N], Tanh)

            # --- DMA stream plan (issue order; deps let HWDGE pipeline) ---
            # leaves in 5 slabs (small first for an early PE start), bias
            # vectors in 3 consolidated slabs issued where consumption allows
            nc.sync.dma_start(hT[8][0][:, 0:1024], leavesT[:, 0, 0:1024])
            nc.sync.dma_start(wsb[:, :8 * 128], wmat[:, :8 * 128])  # mh0 blocks
            nc.sync.dma_start(hT[8][1][:, 0:1024], leavesT[:, 1, 0:1024])
            nc.sync.dma_start(wsb[:, 8 * 128:], wmat[:, 8 * 128:])  # mh1 blocks
            leaf_dma(1024, 3072)                   # chunks 1-2
            vloc_dma(0, 768)                       # bias, chunks 0-2
            leaf_dma(3072, 5120)                   # chunks 3-4
            vloc_dma(768, 1280)                    # bias, chunks 3-4
            leaf_dma(5120, 7168)                   # chunks 5-6
            vloc_dma(1280, 2048)                   # bias, chunks 5-7
            leaf_dma(7168, 8192)                   # chunk 7
            vloc_dma(2048, N_VECS_LOC)             # bias, levels 6..2

            if probe == "A":
                nc.sync.dma_start(out[0:1, 0:16], hT[8][0][:1, :32].bitcast(F32))
            else:
                # level-6 chunks of 128 interleave into the level-7 wavefront
                for c in range(N_L7CHUNKS):
                    do_chunk(7, c * L7CHUNK, L7CHUNK, bias_dve=True)
                    if c >= 3 and c % 2 == 1:      # after c=3,5,7
                        do_chunk(6, 128 * (c - 3) // 2, 128, bias_dve=True)
                do_chunk(6, 384, 128, bias_dve=True)
                # half-chunked cascade: level l's first half depends only on
                # the first half of level l+1, so the chains pipeline
                do_chunk(5, 0, 64, bias_dve=False)
                do_chunk(5, 64, 64, bias_dve=False)
                do_chunk(4, 0, 16, bias_dve=False)
                do_chunk(4, 16, 16, bias_dve=False)
                do_chunk(3, 0, 4, bias_dve=False)
                do_chunk(3, 4, 4, bias_dve=False)
                do_chunk(2, 0, 2, bias_dve=False)

            if probe == "B":
                nc.sync.dma_start(out[0:1, 0:1], t2p[:1, :2].bitcast(F32))
            elif probe != "A":
                # --- AllGather of the level-2 states ---
                cc_in = dram_pool.tile([D, NLOC], mmdt, name="cc_in")
                cc_out = dram_pool.tile([N_CORES * D, NLOC], mmdt,
                                        name="cc_out")
                nc.sync.dma_start(
                    cc_in[:].rearrange("(kh k) n -> k kh n", kh=2),
                    t2p[:].rearrange("k (kh n) -> k kh n", kh=2))
                nc.sync.dma_start(vtail[:], vecs_tailT[:])    # hides under cc
                nc.gpsimd.collective_compute(
                    "AllGather", mybir.AluOpType.bypass,
                    replica_groups=[list(range(N_CORES))],
                    ins=[cc_in.opt()], outs=[cc_out.opt()])
                cc_v = cc_out[:].rearrange("(r kh k) n -> kh k r n",
                                           r=N_CORES, kh=2)
                for kh in (1, 0):
                    nc.sync.dma_start(
                        h2all[kh][:].rearrange("k (r n) -> k r n",
                                               r=N_CORES), cc_v[kh])

                if probe == "C":
                    nc.sync.dma_start(out[0:1, 0:8],
                                      h2all[0][:1, :16].bitcast(F32))
                else:
                    # --- replicated tail: level 1, then the root (x4) ---
                    # kh1 fetched first; kh1 matmuls lead so the kh0 fetch
                    # hides under them
                    rv1 = [h2all[kh][:, :SIZES[2]]
                           .rearrange("k (p four) -> k p four", four=4)
                           for kh in (0, 1)]
                    do_chunk(1, 0, 4, bias_dve=False, vec=vtail,
                             vcol0=0, rview=rv1, hout=h1p, khs=(1, 0))
                    rv0 = [h1p[kh][:, 0:4].unsqueeze(1)
                           .broadcast_to([128, 4, 4]) for kh in (0, 1)]
                    do_chunk(0, 0, 4, bias_dve=False, vec=vtail,
                             vcol0=4, rview=rv0,
                             hout=[t0p[:, 0:4], t0p[:, 4:8]])
                    # root natural-layout store: [128,2] partition-scatter
                    nc.sync.dma_start(
                        out[:].rearrange("o (mh m) -> m (o mh)", mh=2),
                        t0p[:].rearrange("k (mh n) -> k mh n", mh=2)[:, :, 0])

    nc.finalize()
    _BASS[key] = nc
    return nc


def _prep_inputs(vectors, Wl, Wr, mode="fp16"):
    vectors = np.asarray(vectors, dtype=np.float32)
    Wl = np.asarray(Wl, dtype=np.float32)
    Wr = np.asarray(Wr, dtype=np.float32)

    ind = np.arange(1, B + 1, dtype=np.float32)
    lc = (B - ind) / (B - 1)
    rc = (ind - 1) / (B - 1)
    # W_t[i] = W_i.T laid out [k', (mh, i, kh) blocks] for SBUF [128, 2048]
    Wt = np.stack([lc[i] * Wl.T + rc[i] * Wr.T for i in range(B)])  # [4,256k,256m]
    W5 = Wt.reshape(4, 2, 128, 2, 128)            # [i, kh, k', mh, m']
    blocks = [W5[i, kh, :, mh, :]
              for mh in (0, 1) for i in range(4) for kh in (0, 1)]
    wmat = np.ascontiguousarray(np.concatenate(blocks, axis=1),
                                dtype=np.float32)

    # one transposed copy of the node array; per-core slices are views into
    # it laid out [128, 2, n] (part of sharding, no arithmetic)
    vecsT = np.ascontiguousarray(vectors.T)                      # [256, 87381]
    vP = vecsT.reshape(2, 128, vecsT.shape[1])                   # [mh/kh,128,n]

    def packed(cols):
        return np.ascontiguousarray(vP[:, :, cols].transpose(1, 0, 2))

    tail_cols = np.array([1, 2, 3, 4, 0, 0, 0, 0])
    vecs_tailT = packed(tail_cols)

    hdt = np.float16 if mode == "fp16" else np.float32
    in_maps = []
    for c in range(N_CORES):
        o8 = int(OFFSETS[8])
        leaf_cols = slice(o8 + c * N_LEAF_CORE, o8 + (c + 1) * N_LEAF_CORE)
        loc_parts = []
        for l in LOC_LEVELS:
            npl = LOC_PAR[l]
            o = int(OFFSETS[l])
            loc_parts.append(vP[:, :, o + c * npl: o + (c + 1) * npl])
        vloc_c = np.ascontiguousarray(
            np.concatenate(loc_parts, axis=2).transpose(1, 0, 2))
        im = {
            "leavesT": packed(leaf_cols).astype(hdt),
            "vecs_locT": vloc_c.astype(hdt),
            "vecs_tailT": vecs_tailT.astype(hdt),
            "wmat": wmat.astype(hdt),
        }
        in_maps.append(im)
    return in_maps


def kernel(vectors, Wl, Wr, branching, n_levels, _mode="fp16"):
    _lazy_imports()
    assert int(branching) == B and int(n_levels) == L
    vectors = np.asarray(vectors)
    assert vectors.shape == (int(OFFSETS[L]), D), vectors.shape

    nc = _build_nc(mode=_mode)
    in_maps = _prep_inputs(vectors, Wl, Wr, mode=_mode)
    try:
        res = run_bass_kernel_spmd(nc, in_maps, core_ids=list(range(N_CORES)),
                                   trace=False)
    except Exception:
        # transient device hiccups (e.g. NRT_EXEC_UNIT_UNRECOVERABLE right
        # after another process released the cores) clear on a retry
        res = run_bass_kernel_spmd(nc, in_maps, core_ids=list(range(N_CORES)),
                                   trace=False)
    root = res.results[0]["out"]
    return np.asarray(root, dtype=np.float32).reshape(1, D)
